# revision 10
# baseline (speedup 1.0000x reference)
"""Distributed 1D attention kernel for Trainium2 (8 NeuronCores).

Problem: x [4,256,2048], y [4,256,2048] ->
  q = Wq@x, k = Wk@y, v = Wv@y  (per-head d=128, H=8 heads)
  out = Wo @ concat_h(softmax(q^T k / sqrt(128)) applied to v)   -> [4,128,2048]

Sharding: core = 2*b + g where b in [0,4) is the batch and g in {0,1} picks
heads [4g, 4g+4). Each core computes its 4 (b,h) attention pairs plus the
partial Wo projection for its head group; the host sums the two partials
per batch.

Device-side schedule (per core): 16 slots (head h, x-block of 512), the
ScalarE exp stream is the pace-setter (~1.23us per [128,1024] exp, 8 per
slot).  Window s runs A(s) = QK matmuls + exp interleaved on PE with
B(s-1) = {pair-sums on GPSIMD+DVE, denominator tree -> 2 ones-matmuls,
AV accumulation, Newton reciprocal (bit-trick seed read straight from the
PSUM f32 high bytes + 1 NR in bf16), normalize}.  The interleaving order
matches data readiness so no engine head-of-line-blocks another.  The last
two slots use direct ones-matmul denominators (PE is idle at the tail, the
GPSIMD/DVE tree would serialize after the final exp).  Projections are
spread into PE slack; DMAs are split so the first slot starts ~14us in
(7.5us of that is fixed NRT preamble).
"""

import sys

if "/opt/trn_rl_repo" not in sys.path:
    sys.path.insert(0, "/opt/trn_rl_repo")

import numpy as np
import ml_dtypes


def _install_ntff_shim():
    """antenv.axon_hooks is absent from this image, which crashes
    run_bass_kernel_spmd(trace=True). Recreate it from the hook factory
    that trn_agent_boot ships."""
    import types

    if "antenv.axon_hooks" in sys.modules:
        return
    mod = types.ModuleType("antenv.axon_hooks")
    _hook = [None]
    mod.set_axon_ntff_profile_hook = lambda h: _hook.__setitem__(0, h)
    mod.get_axon_ntff_profile_hook = lambda: _hook[0]
    sys.modules["antenv.axon_hooks"] = mod
    try:
        import antenv

        antenv.axon_hooks = mod
    except ImportError:
        pass
    try:
        from trn_agent_boot.trn_boot import _ntff_profile_via_ctypes

        mod.set_axon_ntff_profile_hook(
            _ntff_profile_via_ctypes("/opt/axon/libaxon_pjrt.so")
        )
    except Exception:
        pass


_install_ntff_shim()

import concourse.bass as bass
import concourse.mybir as mybir
import concourse.tile as tile
from concourse.bass_utils import run_bass_kernel_spmd

B, C, N, H, D = 4, 256, 2048, 8, 128
HPC = H // 2  # heads per core
NCORES = 8
BF = mybir.dt.bfloat16
F32 = mybir.dt.float32
I16 = mybir.dt.int16
NYT = N // 128  # 16 y tiles
NXB = N // 512  # 4 x blocks
SCALE = 1.0 / float(np.sqrt(D))
MAGIC = 0x7EF1  # bf16 reciprocal bit-trick constant

LAST_EXEC_NS = None
LAST_RESULTS = None


def _split_multi_waits(nc):
    """This walrus build accepts at most ONE sync wait per instruction;
    Tile's semaphore assignment attaches several. Hoist the extras into
    standalone event-semaphore instructions on the same engine."""
    ctr = 0
    for fn in nc.m.functions:
        for blk in fn.blocks:
            new_list = []
            changed = False
            for inst in blk.instructions:
                si = inst.sync_info
                if si is not None and len(si.on_wait) > 1:
                    waits = list(si.on_wait)
                    ups = list(si.on_update)
                    for w in waits[:-1]:
                        ev = mybir.InstEventSemaphore(
                            name=f"waitsplit-{ctr}", ins=[], outs=[]
                        )
                        ctr += 1
                        ev.engine = inst.engine
                        ev.sync_info = mybir.SyncInfo(on_wait=[w], on_update=[])
                        new_list.append(ev)
                    inst.sync_info = mybir.SyncInfo(on_wait=[waits[-1]], on_update=ups)
                    changed = True
                new_list.append(inst)
            if changed:
                blk.instructions = new_list
    return ctr


def _build_nc():
    nc = bass.Bass("TRN2", target_bir_lowering=False, debug=False)

    xb = nc.dram_tensor("xb", [C, N], BF, kind="ExternalInput")
    yb = nc.dram_tensor("yb", [C, N], BF, kind="ExternalInput")
    # wpack = [WKT | WQT | WVT] along the output dim, [c, 3*hd]
    wpack = nc.dram_tensor("wpack", [C, 3 * HPC * D], BF, kind="ExternalInput")
    wot = nc.dram_tensor("wot", [HPC * D, D], BF, kind="ExternalInput")  # [hd, o]
    out = nc.dram_tensor("out", [D, N], F32, kind="ExternalOutput")

    EXPF = mybir.ActivationFunctionType.Exp
    SUB = mybir.AluOpType.subtract
    MUL = mybir.AluOpType.mult

    with tile.TileContext(nc) as tc:
        with (
            tc.tile_pool(name="w", bufs=1) as wpool,
            tc.tile_pool(name="big", bufs=1) as bigpool,
            tc.tile_pool(name="e", bufs=20) as epool,
            tc.tile_pool(name="s", bufs=12) as spool,
            tc.tile_pool(name="uw", bufs=8) as uwpool,
            tc.tile_pool(name="nwt", bufs=8) as nwtpool,
            tc.tile_pool(name="att", bufs=4) as attpool,
            tc.tile_pool(name="small", bufs=4) as smallpool,
            tc.tile_pool(name="pl", bufs=2, space="PSUM") as plpool,
            tc.tile_pool(name="po", bufs=2, space="PSUM") as popool,
            tc.tile_pool(name="pd", bufs=2, space="PSUM") as pdpool,
        ):
            # ---- constants + ACT exp-table preload at t=0 -------------------
            ONES = wpool.tile([128, 128], BF, tag="ONES")
            nc.gpsimd.memset(ONES[:], 1.0)
            TMP = wpool.tile([128, 32], BF, tag="TMP")
            nc.gpsimd.memset(TMP[:, 0:16], 0.0)
            # loads the exp_and_others table set (~2.7us) while DMAs run
            nc.scalar.activation(TMP[:, 16:32], TMP[:, 0:16], EXPF)

            # ---- input loads, ordered for earliest first QK slot -----------
            xr = xb.rearrange("(kt p) n -> p kt n", p=128)
            yr = yb.rearrange("(kt p) n -> p kt n", p=128)
            wpr = wpack.rearrange("(kt p) m -> p kt m", p=128)
            WP = wpool.tile([128, 2, 3 * HPC * D], BF, tag="WP")
            nc.sync.dma_start(WP[:, :, 0 : HPC * D], wpr[:, :, 0 : HPC * D])  # WKT
            nc.sync.dma_start(
                WP[:, :, HPC * D : 2 * HPC * D], wpr[:, :, HPC * D : 2 * HPC * D]
            )  # WQT
            X = bigpool.tile([128, 2, N], BF, tag="X")
            nc.sync.dma_start(X[:, :, 0:512], xr[:, :, 0:512])
            Y = bigpool.tile([128, 2, N], BF, tag="Y")
            for q in range(4):
                qs = slice(q * 512, (q + 1) * 512)
                nc.sync.dma_start(Y[:, :, qs], yr[:, :, qs])
            nc.sync.dma_start(
                WP[:, :, 2 * HPC * D : 3 * HPC * D],
                wpr[:, :, 2 * HPC * D : 3 * HPC * D],
            )  # WVT
            nc.sync.dma_start(X[:, :, 512:N], xr[:, :, 512:N])
            WOT = wpool.tile([128, HPC, D], BF, tag="WOT")
            nc.sync.dma_start(WOT[:], wot.rearrange("(h p) o -> p h o", p=128))
            WKT = WP[:, :, 0 : HPC * D]
            WQT = WP[:, :, HPC * D : 2 * HPC * D]
            WVT = WP[:, :, 2 * HPC * D : 3 * HPC * D]

            # HAM warm-up: keep the PE clock-gate open while input DMAs run.
            WARM = plpool.tile([128, 1024], F32, tag="pl", name="warm")
            for _wi in range(16):
                nc.tensor.matmul(
                    WARM[:, :128], ONES[:], ONES[:], start=True, stop=True
                )

            # ---- projections ------------------------------------------------
            Q = bigpool.tile([128, HPC, N], BF, tag="Q")
            K = bigpool.tile([128, HPC, N], BF, tag="K")
            VT = bigpool.tile([128, NYT, HPC * D], BF, tag="VT")

            def proj_k(h):
                hs = slice(h * 128, (h + 1) * 128)
                for nb in range(NXB):
                    ns = slice(nb * 512, (nb + 1) * 512)
                    pk = pdpool.tile([128, 512], F32, tag="pd", name=f"pk_{h}_{nb}")
                    nc.tensor.matmul(
                        pk[:], WKT[:, 0, hs], Y[:, 0, ns], start=True, stop=False
                    )
                    nc.tensor.matmul(
                        pk[:], WKT[:, 1, hs], Y[:, 1, ns], start=False, stop=True
                    )
                    nc.vector.tensor_copy(K[:, h, ns], pk[:])

            def proj_q(h, nb0, nb1):
                hs = slice(h * 128, (h + 1) * 128)
                for nb in range(nb0, nb1):
                    ns = slice(nb * 512, (nb + 1) * 512)
                    ps = pdpool.tile([128, 512], F32, tag="pd", name=f"pq_{h}_{nb}")
                    nc.tensor.matmul(
                        ps[:], WQT[:, 0, hs], X[:, 0, ns], start=True, stop=False
                    )
                    nc.tensor.matmul(
                        ps[:], WQT[:, 1, hs], X[:, 1, ns], start=False, stop=True
                    )
                    nc.vector.tensor_copy(Q[:, h, ns], ps[:])

            def proj_v(yt0, yt1):
                for yt in range(yt0, yt1):
                    ys = slice(yt * 128, (yt + 1) * 128)
                    pv = pdpool.tile([128, 512], F32, tag="pd", name=f"pv_{yt}")
                    nc.tensor.matmul(
                        pv[:], Y[:, 0, ys], WVT[:, 0, :], start=True, stop=False
                    )
                    nc.tensor.matmul(
                        pv[:], Y[:, 1, ys], WVT[:, 1, :], start=False, stop=True
                    )
                    nc.vector.tensor_copy(VT[:, yt, :], pv[:])

            # ---- attention: slots (h, xblk), depth-1 interleaved pipeline --
            slots = [(h, xblk) for h in range(HPC) for xblk in range(NXB)]
            NS = len(slots)
            E_t, S_t, W_t, po_t, pd_t = {}, {}, {}, {}, {}
            att_tiles = {}
            # proj chunks emitted inside window s (PE slack scheduling)
            proj_sched = {
                0: [lambda: proj_v(0, NYT), lambda: proj_q(0, 1, NXB)],
                1: [lambda: proj_k(1)],
                2: [lambda: proj_q(1, 0, NXB)],
                5: [lambda: proj_k(2)],
                6: [lambda: proj_q(2, 0, NXB)],
                9: [lambda: proj_k(3)],
                10: [lambda: proj_q(3, 0, NXB)],
            }

            def emit_qk(s, g):
                h, xblk = slots[s]
                xs = slice(xblk * 512, (xblk + 1) * 512)
                pl = plpool.tile([128, 1024], F32, tag="pl", name=f"pl_{s}_{g}")
                for j in range(2):
                    yt = 2 * g + j
                    nc.tensor.matmul(
                        pl[:, j * 512 : (j + 1) * 512],
                        K[:, h, yt * 128 : (yt + 1) * 128],
                        Q[:, h, xs],
                        start=True,
                        stop=True,
                    )
                nc.scalar.activation(E_t[s][g][:], pl[:], EXPF, scale=SCALE)

            def emit_av(s, yt0, yt1):
                h, _ = slots[s]
                E = E_t[s]
                for yt in range(yt0, yt1):
                    nc.tensor.matmul(
                        po_t[s][:],
                        VT[:, yt, h * 128 : (h + 1) * 128],
                        E[yt // 2][:, yt % 2, :],
                        start=(yt == 0),
                        stop=(yt == NYT - 1),
                    )

            rc_t = {}

            def emit_newton(s):
                # Newton reciprocal: rc ~= 1/den, seed from PSUM f32 hi-bytes
                pdt = pd_t[s]
                r0 = nwtpool.tile([128, 512], BF, tag="nwt", name=f"r0_{s}")
                nc.vector.tensor_scalar(
                    r0[:].bitcast(I16), pdt[:].bitcast(I16)[:, 1::2], MAGIC, -1,
                    SUB, MUL,
                )
                tt = nwtpool.tile([128, 512], BF, tag="nwt", name=f"t_{s}")
                nc.vector.tensor_tensor(tt[:], pdt[:], r0[:], MUL)
                uu = nwtpool.tile([128, 512], BF, tag="nwt", name=f"u_{s}")
                nc.vector.tensor_scalar(uu[:], tt[:], 2.0, -1.0, SUB, MUL)
                rc = nwtpool.tile([128, 512], BF, tag="nwt", name=f"rc_{s}")
                nc.vector.tensor_tensor(rc[:], r0[:], uu[:], MUL)
                rc_t[s] = rc

            def emit_mul(s):
                h, xblk = slots[s]
                if h == 0:
                    att_tiles[xblk] = attpool.tile(
                        [128, HPC, 512], BF, tag="ATT", name=f"ATT_{xblk}"
                    )
                nc.vector.tensor_tensor(
                    att_tiles[xblk][:, h, :], po_t[s][:], rc_t.pop(s)[:], MUL
                )

            def emit_wo(s):
                h, xblk = slots[s]
                xs = slice(xblk * 512, (xblk + 1) * 512)
                ATT = att_tiles[xblk]
                pw = pdpool.tile([128, 512], F32, tag="pd", name=f"pw_{xblk}")
                for hh in range(HPC):
                    nc.tensor.matmul(
                        pw[:],
                        WOT[:, hh, :],
                        ATT[:, hh, :],
                        start=(hh == 0),
                        stop=(hh == HPC - 1),
                    )
                ob = smallpool.tile([128, 512], F32, tag="osb", name=f"ob_{xblk}")
                nc.vector.tensor_copy(ob[:], pw[:])
                nc.sync.dma_start(out[:, xs], ob[:])

            def emit_s(s, g):
                # pair-sum for (s, g), emitted right after its exp so GPSIMD
                # starts during window s instead of serializing in s+1
                if s not in S_t:
                    S_t[s] = [
                        spool.tile([128, 512], BF, tag="S", name=f"S_{s}_{g2}")
                        for g2 in range(8)
                    ]
                E, S = E_t[s], S_t[s]
                eng = nc.gpsimd if g % 2 == 0 else nc.vector
                eng.tensor_add(S[g][:], E[g][:, 0, :], E[g][:, 1, :])

            proj_k(0)
            proj_q(0, 0, 1)

            for w in range(NS + 1):
                s = w if w < NS else None  # A-phase slot
                p = w - 1  # B-phase slot
                if s is not None:
                    E_t[s] = [
                        epool.tile([128, 2, 512], BF, tag="E", name=f"E_{s}_{g}")
                        for g in range(8)
                    ]
                if p >= 0:
                    po_t[p] = popool.tile([128, 512], F32, tag="po", name=f"pav_{p}")
                    pd_t[p] = pdpool.tile([128, 512], F32, tag="pd", name=f"pden_{p}")
                    pdt = pd_t[p]
                    # finish the denominator tree and start den matmuls EARLY
                    # (inputs were produced during window p)
                    if p < NS - 2:
                        U = W_t[p]
                        W = [
                            uwpool.tile([128, 512], BF, tag="UW", name=f"W_{p}_{i}")
                            for i in range(2)
                        ]
                        nc.vector.tensor_add(W[0][:], U[0][:], U[1][:])
                        nc.vector.tensor_add(W[1][:], U[2][:], U[3][:])
                        nc.tensor.matmul(pdt[:], ONES[:], W[0][:], start=True, stop=False)
                        nc.tensor.matmul(pdt[:], ONES[:], W[1][:], start=False, stop=True)
                    elif p == NS - 2:
                        S = S_t[p]
                        for g in range(8):
                            nc.tensor.matmul(
                                pdt[:], ONES[:], S[g][:], start=(g == 0), stop=(g == 7)
                            )
                    else:  # last slot: direct ones-matmuls on E, chunk-gated
                        E = E_t[p]
                        for g in range(8):
                            for j in range(2):
                                nc.tensor.matmul(
                                    pdt[:],
                                    ONES[:],
                                    E[g][:, j, :],
                                    start=(g == 0 and j == 0),
                                    stop=(g == 7 and j == 1),
                                )
                            emit_av(p, 2 * g, 2 * g + 2)
                    emit_newton(p)
                # A: QK+exp trickle with in-window pair-sums, interleaved
                # with B(p)'s AV chunks and proj chunks
                if s is not None:
                    emit_qk(s, 0)
                    if s < NS - 1:
                        emit_s(s, 0)
                    emit_qk(s, 1)
                    if s < NS - 1:
                        emit_s(s, 1)
                if w == 0:
                    for fn in proj_sched.get(w, []):
                        fn()
                if p >= 0 and p < NS - 1:
                    emit_av(p, 0, 3)
                if s is not None:
                    emit_qk(s, 2)
                    if s < NS - 1:
                        emit_s(s, 2)
                if p >= 0 and p < NS - 1:
                    emit_av(p, 3, 6)
                if s is not None:
                    emit_qk(s, 3)
                    if s < NS - 1:
                        emit_s(s, 3)
                if p >= 0 and p < NS - 1:
                    emit_av(p, 6, 9)
                if s is not None:
                    emit_qk(s, 4)
                    if s < NS - 1:
                        emit_s(s, 4)
                if p >= 0 and p < NS - 1:
                    emit_av(p, 9, 12)
                if w > 0:
                    for fn in proj_sched.get(w, []):
                        fn()
                if s is not None:
                    emit_qk(s, 5)
                    if s < NS - 1:
                        emit_s(s, 5)
                if p >= 0 and p < NS - 1:
                    emit_av(p, 12, 15)
                if s is not None:
                    emit_qk(s, 6)
                    if s < NS - 1:
                        emit_s(s, 6)
                    emit_qk(s, 7)
                    if s < NS - 1:
                        emit_s(s, 7)
                if p >= 0 and p < NS - 1:
                    emit_av(p, 15, 16)
                # tree level U for slot s (S tiles just written); slot 14
                # uses its S tiles directly, slot 15 has no tree at all
                if s is not None and s < NS - 2:
                    S = S_t[s]
                    U = [
                        uwpool.tile([128, 512], BF, tag="UW", name=f"U_{s}_{i}")
                        for i in range(4)
                    ]
                    nc.gpsimd.tensor_add(U[0][:], S[0][:], S[1][:])
                    nc.gpsimd.tensor_add(U[1][:], S[2][:], S[3][:])
                    nc.vector.tensor_add(U[2][:], S[4][:], S[5][:])
                    nc.vector.tensor_add(U[3][:], S[6][:], S[7][:])
                    W_t[s] = U
                if p >= 0:
                    # normalize (DVE) then Wo partial at the last head
                    emit_mul(p)
                    if slots[p][0] == HPC - 1:
                        emit_wo(p)
                    E_t.pop(p)
                    S_t.pop(p, None)
                    W_t.pop(p, None)

    _split_multi_waits(nc)
    return nc


_NC = None


def _get_nc():
    global _NC
    if _NC is None:
        _NC = _build_nc()
    return _NC


def kernel(x, y, Wq, Wk, Wv, Wo):
    global LAST_EXEC_NS, LAST_RESULTS
    x = np.asarray(x, dtype=np.float32)
    y = np.asarray(y, dtype=np.float32)
    Wq3 = np.asarray(Wq, dtype=np.float32).reshape(H, D, C)
    Wk3 = np.asarray(Wk, dtype=np.float32).reshape(H, D, C)
    Wv3 = np.asarray(Wv, dtype=np.float32).reshape(H, D, C)
    Wo2 = np.asarray(Wo, dtype=np.float32)  # [D, H*D]

    bf16 = ml_dtypes.bfloat16

    in_maps = []
    for core in range(NCORES):
        b, g = core // 2, core % 2
        hsl = slice(4 * g, 4 * g + HPC)
        wqt = Wq3[hsl].reshape(HPC * D, C).T  # [c, hd]
        wkt = Wk3[hsl].reshape(HPC * D, C).T
        wvt = Wv3[hsl].reshape(HPC * D, C).T
        wot = Wo2[:, 4 * g * D : (4 * g + HPC) * D].T  # [hd, o]
        wpack = np.concatenate([wkt, wqt, wvt], axis=1)  # [c, 3*hd]
        in_maps.append(
            {
                "xb": np.ascontiguousarray(x[b]).astype(bf16),
                "yb": np.ascontiguousarray(y[b]).astype(bf16),
                "wpack": np.ascontiguousarray(wpack).astype(bf16),
                "wot": np.ascontiguousarray(wot).astype(bf16),
            }
        )

    import os

    trace = bool(int(os.environ.get("ATTN_TRACE", "0")))
    res = run_bass_kernel_spmd(
        _get_nc(), in_maps, core_ids=list(range(NCORES)), trace=trace
    )
    LAST_EXEC_NS = res.exec_time_ns
    LAST_RESULTS = res

    out = np.empty((B, D, N), dtype=np.float32)
    for b in range(B):
        out[b] = res.results[2 * b]["out"] + res.results[2 * b + 1]["out"]
    return out


# revision 15
# speedup vs baseline: 1.0429x; 1.0429x over previous
"""Distributed 1D attention kernel for Trainium2 (8 NeuronCores).

Problem: x [4,256,2048], y [4,256,2048] ->
  q = Wq@x, k = Wk@y, v = Wv@y  (per-head d=128, H=8 heads)
  out = Wo @ concat_h(softmax(q^T k / sqrt(128)) applied to v)   -> [4,128,2048]

Sharding: core = 2*b + g where b in [0,4) is the batch and g in {0,1} picks
heads [4g, 4g+4). Each core computes its 4 (b,h) attention pairs plus the
partial Wo projection for its head group; the host sums the two partials
per batch.

Device-side schedule (per core): 16 slots (head h, x-block of 512), the
ScalarE exp stream is the pace-setter (~1.23us per [128,1024] exp, 8 per
slot).  Window s runs A(s) = QK matmuls + exp interleaved on PE with
B(s-1) = {pair-sums on GPSIMD+DVE, denominator tree -> 2 ones-matmuls,
AV accumulation, Newton reciprocal (bit-trick seed read straight from the
PSUM f32 high bytes + 1 NR in bf16), normalize}.  The interleaving order
matches data readiness so no engine head-of-line-blocks another.  The last
two slots use direct ones-matmul denominators (PE is idle at the tail, the
GPSIMD/DVE tree would serialize after the final exp).  Projections are
spread into PE slack; DMAs are split so the first slot starts ~14us in
(7.5us of that is fixed NRT preamble).
"""

import sys

if "/opt/trn_rl_repo" not in sys.path:
    sys.path.insert(0, "/opt/trn_rl_repo")

import numpy as np
import ml_dtypes


def _install_ntff_shim():
    """antenv.axon_hooks is absent from this image, which crashes
    run_bass_kernel_spmd(trace=True). Recreate it from the hook factory
    that trn_agent_boot ships."""
    import types

    if "antenv.axon_hooks" in sys.modules:
        return
    mod = types.ModuleType("antenv.axon_hooks")
    _hook = [None]
    mod.set_axon_ntff_profile_hook = lambda h: _hook.__setitem__(0, h)
    mod.get_axon_ntff_profile_hook = lambda: _hook[0]
    sys.modules["antenv.axon_hooks"] = mod
    try:
        import antenv

        antenv.axon_hooks = mod
    except ImportError:
        pass
    try:
        from trn_agent_boot.trn_boot import _ntff_profile_via_ctypes

        mod.set_axon_ntff_profile_hook(
            _ntff_profile_via_ctypes("/opt/axon/libaxon_pjrt.so")
        )
    except Exception:
        pass


_install_ntff_shim()

import concourse.bass as bass
import concourse.mybir as mybir
import concourse.tile as tile
from concourse.bass_utils import run_bass_kernel_spmd

B, C, N, H, D = 4, 256, 2048, 8, 128
HPC = H // 2  # heads per core
NCORES = 8
BF = mybir.dt.bfloat16
F32 = mybir.dt.float32
I16 = mybir.dt.int16
NYT = N // 128  # 16 y tiles
NXB = N // 512  # 4 x blocks
SCALE = 1.0 / float(np.sqrt(D))
MAGIC = 0x7EF1  # bf16 reciprocal bit-trick constant

LAST_EXEC_NS = None
LAST_RESULTS = None


def _split_multi_waits(nc):
    """This walrus build accepts at most ONE sync wait per instruction;
    Tile's semaphore assignment attaches several. Hoist the extras into
    standalone event-semaphore instructions on the same engine."""
    ctr = 0
    for fn in nc.m.functions:
        for blk in fn.blocks:
            new_list = []
            changed = False
            for inst in blk.instructions:
                si = inst.sync_info
                if si is not None and len(si.on_wait) > 1:
                    waits = list(si.on_wait)
                    ups = list(si.on_update)
                    for w in waits[:-1]:
                        ev = mybir.InstEventSemaphore(
                            name=f"waitsplit-{ctr}", ins=[], outs=[]
                        )
                        ctr += 1
                        ev.engine = inst.engine
                        ev.sync_info = mybir.SyncInfo(on_wait=[w], on_update=[])
                        new_list.append(ev)
                    inst.sync_info = mybir.SyncInfo(on_wait=[waits[-1]], on_update=ups)
                    changed = True
                new_list.append(inst)
            if changed:
                blk.instructions = new_list
    return ctr


def _build_nc():
    nc = bass.Bass("TRN2", target_bir_lowering=False, debug=False)

    xb = nc.dram_tensor("xb", [C, N], BF, kind="ExternalInput")
    yb = nc.dram_tensor("yb", [C, N], BF, kind="ExternalInput")
    # wpack = [WKT | WQT | WVT] along the output dim, [c, 3*hd]
    wpack = nc.dram_tensor("wpack", [C, 3 * HPC * D], BF, kind="ExternalInput")
    wot = nc.dram_tensor("wot", [HPC * D, D], BF, kind="ExternalInput")  # [hd, o]
    out = nc.dram_tensor("out", [D, N], F32, kind="ExternalOutput")

    EXPF = mybir.ActivationFunctionType.Exp
    SUB = mybir.AluOpType.subtract
    MUL = mybir.AluOpType.mult

    with tile.TileContext(nc) as tc:
        with (
            tc.tile_pool(name="w", bufs=1) as wpool,
            tc.tile_pool(name="big", bufs=1) as bigpool,
            tc.tile_pool(name="e", bufs=20) as epool,
            tc.tile_pool(name="s", bufs=12) as spool,
            tc.tile_pool(name="uw", bufs=14) as uwpool,
            tc.tile_pool(name="nwt", bufs=8) as nwtpool,
            tc.tile_pool(name="att", bufs=4) as attpool,
            tc.tile_pool(name="small", bufs=4) as smallpool,
            tc.tile_pool(name="pl", bufs=2, space="PSUM") as plpool,
            tc.tile_pool(name="po", bufs=2, space="PSUM") as popool,
            tc.tile_pool(name="pd", bufs=1, space="PSUM") as pdpool,
            tc.tile_pool(name="pj", bufs=1, space="PSUM") as pjpool,
        ):
            # ---- constants + ACT exp-table preload at t=0 -------------------
            ONES = wpool.tile([128, 128], BF, tag="ONES")
            nc.gpsimd.memset(ONES[:], 1.0)
            TMP = wpool.tile([128, 32], BF, tag="TMP")
            nc.gpsimd.memset(TMP[:, 0:16], 0.0)
            # loads the exp_and_others table set (~2.7us) while DMAs run
            nc.scalar.activation(TMP[:, 16:32], TMP[:, 0:16], EXPF)

            # ---- input loads, ordered for earliest first QK slot -----------
            xr = xb.rearrange("(kt p) n -> p kt n", p=128)
            yr = yb.rearrange("(kt p) n -> p kt n", p=128)
            wpr = wpack.rearrange("(kt p) m -> p kt m", p=128)
            WP = wpool.tile([128, 2, 3 * HPC * D], BF, tag="WP")
            nc.sync.dma_start(
                WP[:, :, 0 : 2 * HPC * D], wpr[:, :, 0 : 2 * HPC * D]
            )  # WKT+WQT
            X = bigpool.tile([128, 2, N], BF, tag="X")
            nc.sync.dma_start(X[:, :, 0:512], xr[:, :, 0:512])
            Y = bigpool.tile([128, 2, N], BF, tag="Y")
            nc.sync.dma_start(Y[:, :, 0:1024], yr[:, :, 0:1024])
            nc.sync.dma_start(Y[:, :, 1024:N], yr[:, :, 1024:N])
            nc.sync.dma_start(
                WP[:, :, 2 * HPC * D : 3 * HPC * D],
                wpr[:, :, 2 * HPC * D : 3 * HPC * D],
            )  # WVT
            nc.sync.dma_start(X[:, :, 512:N], xr[:, :, 512:N])
            WOT = wpool.tile([128, HPC, D], BF, tag="WOT")
            nc.sync.dma_start(WOT[:], wot.rearrange("(h p) o -> p h o", p=128))
            WKT = WP[:, :, 0 : HPC * D]
            WQT = WP[:, :, HPC * D : 2 * HPC * D]
            WVT = WP[:, :, 2 * HPC * D : 3 * HPC * D]

            # HAM warm-up: keep the PE clock-gate open while input DMAs run.
            WARM = plpool.tile([128, 1024], F32, tag="pl", name="warm")
            for _wi in range(56):
                nc.tensor.matmul(
                    WARM[:, :128], ONES[:], ONES[:], start=True, stop=True
                )

            # ---- projections ------------------------------------------------
            Q = bigpool.tile([128, HPC, N], BF, tag="Q")
            K = bigpool.tile([128, HPC, N], BF, tag="K")
            VT = bigpool.tile([128, NYT, HPC * D], BF, tag="VT")

            # proj psum tiles alternate between the pj and pd pools (1 bank
            # each, prompt cast readers) for a 2-wide trickle
            _projalt = [0]

            def _proj_tile(name):
                _projalt[0] ^= 1
                pool = pjpool if _projalt[0] else pdpool
                tag = "pj" if _projalt[0] else "pd"
                return pool.tile([128, 512], F32, tag=tag, name=name)

            def proj_k(h):
                hs = slice(h * 128, (h + 1) * 128)
                for nb in range(NXB):
                    ns = slice(nb * 512, (nb + 1) * 512)
                    pk = _proj_tile(f"pk_{h}_{nb}")
                    nc.tensor.matmul(
                        pk[:], WKT[:, 0, hs], Y[:, 0, ns], start=True, stop=False
                    )
                    nc.tensor.matmul(
                        pk[:], WKT[:, 1, hs], Y[:, 1, ns], start=False, stop=True
                    )
                    nc.vector.tensor_copy(K[:, h, ns], pk[:])

            def proj_q(h, nb0, nb1):
                hs = slice(h * 128, (h + 1) * 128)
                for nb in range(nb0, nb1):
                    ns = slice(nb * 512, (nb + 1) * 512)
                    ps = _proj_tile(f"pq_{h}_{nb}")
                    nc.tensor.matmul(
                        ps[:], WQT[:, 0, hs], X[:, 0, ns], start=True, stop=False
                    )
                    nc.tensor.matmul(
                        ps[:], WQT[:, 1, hs], X[:, 1, ns], start=False, stop=True
                    )
                    nc.vector.tensor_copy(Q[:, h, ns], ps[:])

            def proj_v(yt0, yt1):
                for yt in range(yt0, yt1):
                    ys = slice(yt * 128, (yt + 1) * 128)
                    pv = _proj_tile(f"pv_{yt}")
                    nc.tensor.matmul(
                        pv[:], Y[:, 0, ys], WVT[:, 0, :], start=True, stop=False
                    )
                    nc.tensor.matmul(
                        pv[:], Y[:, 1, ys], WVT[:, 1, :], start=False, stop=True
                    )
                    nc.vector.tensor_copy(VT[:, yt, :], pv[:])

            # ---- attention: slots (h, xblk), depth-1 interleaved pipeline --
            slots = [(h, xblk) for h in range(HPC) for xblk in range(NXB)]
            NS = len(slots)
            E_t, S_t, W_t, po_t, pd_t = {}, {}, {}, {}, {}
            pending_wo = []
            att_tiles = {}
            # proj chunks emitted inside window s (PE slack scheduling)
            proj_sched = {
                2: [lambda: proj_k(1)],
                3: [lambda: proj_q(1, 0, NXB)],
                6: [lambda: proj_k(2)],
                7: [lambda: proj_q(2, 0, NXB)],
                10: [lambda: proj_k(3)],
                11: [lambda: proj_q(3, 0, NXB)],
            }
            # window-0 proj (V + rest of Q0) is spread between the qk
            # emissions directly in the loop below
            w0_proj = [
                lambda: proj_v(0, 4),
                lambda: proj_v(4, 8),
                lambda: proj_v(8, 12),
                lambda: proj_v(12, NYT),
                lambda: proj_q(0, 1, NXB),
            ]

            def emit_qk(s, g):
                h, xblk = slots[s]
                xs = slice(xblk * 512, (xblk + 1) * 512)
                pl = plpool.tile([128, 1024], F32, tag="pl", name=f"pl_{s}_{g}")
                for j in range(2):
                    yt = 2 * g + j
                    nc.tensor.matmul(
                        pl[:, j * 512 : (j + 1) * 512],
                        K[:, h, yt * 128 : (yt + 1) * 128],
                        Q[:, h, xs],
                        start=True,
                        stop=True,
                    )
                nc.scalar.activation(E_t[s][g][:], pl[:], EXPF, scale=SCALE)

            def emit_av(s, yt0, yt1):
                h, _ = slots[s]
                E = E_t[s]
                for yt in range(yt0, yt1):
                    nc.tensor.matmul(
                        po_t[s][:],
                        VT[:, yt, h * 128 : (h + 1) * 128],
                        E[yt // 2][:, yt % 2, :],
                        start=(yt == 0),
                        stop=(yt == NYT - 1),
                    )

            rc_t = {}

            def emit_newton(s):
                # Newton reciprocal: rc ~= 1/den, seed from PSUM f32 hi-bytes
                pdt = pd_t[s]
                r0 = nwtpool.tile([128, 512], BF, tag="nwt", name=f"r0_{s}")
                nc.vector.tensor_scalar(
                    r0[:].bitcast(I16), pdt[:].bitcast(I16)[:, 1::2], MAGIC, -1,
                    SUB, MUL,
                )
                tt = nwtpool.tile([128, 512], BF, tag="nwt", name=f"t_{s}")
                nc.vector.tensor_tensor(tt[:], pdt[:], r0[:], MUL)
                uu = nwtpool.tile([128, 512], BF, tag="nwt", name=f"u_{s}")
                nc.vector.tensor_scalar(uu[:], tt[:], 2.0, -1.0, SUB, MUL)
                rc = nwtpool.tile([128, 512], BF, tag="nwt", name=f"rc_{s}")
                nc.vector.tensor_tensor(rc[:], r0[:], uu[:], MUL)
                rc_t[s] = rc

            def emit_mul(s):
                h, xblk = slots[s]
                if h == 0:
                    att_tiles[xblk] = attpool.tile(
                        [128, HPC, 512], BF, tag="ATT", name=f"ATT_{xblk}"
                    )
                nc.vector.tensor_tensor(
                    att_tiles[xblk][:, h, :], po_t[s][:], rc_t.pop(s)[:], MUL
                )

            def emit_wo(s):
                h, xblk = slots[s]
                xs = slice(xblk * 512, (xblk + 1) * 512)
                ATT = att_tiles[xblk]
                pw = _proj_tile(f"pw_{xblk}")
                for hh in range(HPC):
                    nc.tensor.matmul(
                        pw[:],
                        WOT[:, hh, :],
                        ATT[:, hh, :],
                        start=(hh == 0),
                        stop=(hh == HPC - 1),
                    )
                ob = smallpool.tile([128, 512], F32, tag="osb", name=f"ob_{xblk}")
                nc.vector.tensor_copy(ob[:], pw[:])
                nc.sync.dma_start(out[:, xs], ob[:])

            def emit_s(s, g):
                # pair-sum for (s, g), emitted right after its exp so GPSIMD
                # starts during window s instead of serializing in s+1
                if s not in S_t:
                    S_t[s] = [
                        spool.tile([128, 512], BF, tag="S", name=f"S_{s}_{g2}")
                        for g2 in range(8)
                    ]
                E, S = E_t[s], S_t[s]
                eng = nc.gpsimd if g % 2 == 0 else nc.vector
                eng.tensor_add(S[g][:], E[g][:, 0, :], E[g][:, 1, :])

            proj_k(0)
            proj_q(0, 0, 1)

            for w in range(NS + 1):
                s = w if w < NS else None  # A-phase slot
                p = w - 1  # B-phase slot
                if s is not None:
                    E_t[s] = [
                        epool.tile([128, 2, 512], BF, tag="E", name=f"E_{s}_{g}")
                        for g in range(8)
                    ]
                if p >= 0:
                    po_t[p] = popool.tile([128, 512], F32, tag="po", name=f"pav_{p}")
                if p == NS - 1:
                    # tail: direct ones-matmul denominator on E, chunk-gated
                    # behind the last exp stream, interleaved with AV
                    pd_t[p] = pdpool.tile([128, 512], F32, tag="pd", name=f"pden_{p}")
                    E = E_t[p]
                    for g in range(8):
                        for j in range(2):
                            nc.tensor.matmul(
                                pd_t[p][:],
                                ONES[:],
                                E[g][:, j, :],
                                start=(g == 0 and j == 0),
                                stop=(g == 7 and j == 1),
                            )
                        emit_av(p, 2 * g, 2 * g + 2)
                    if pending_wo:
                        emit_wo(pending_wo.pop(0))
                    emit_newton(p)
                # A: QK+exp trickle with in-window pair-sums, interleaved
                # with B(p)'s AV chunks and proj chunks
                if s is not None:
                    emit_qk(s, 0)
                    if s < NS - 1:
                        emit_s(s, 0)
                    emit_qk(s, 1)
                    if s < NS - 1:
                        emit_s(s, 1)
                if w == 0:
                    w0_proj[0]()
                if p >= 0 and p < NS - 1:
                    emit_av(p, 0, 3)
                if s is not None:
                    emit_qk(s, 2)
                    if s < NS - 1:
                        emit_s(s, 2)
                if w == 0:
                    w0_proj[1]()
                if p >= 0 and p < NS - 1:
                    emit_av(p, 3, 6)
                if s is not None:
                    emit_qk(s, 3)
                    if s < NS - 1:
                        emit_s(s, 3)
                if w == 0:
                    w0_proj[2]()
                if p >= 0 and p < NS - 1:
                    emit_av(p, 6, 9)
                if s is not None:
                    emit_qk(s, 4)
                    if s < NS - 1:
                        emit_s(s, 4)
                if w == 0:
                    w0_proj[3]()
                if p >= 0 and p < NS - 1:
                    emit_av(p, 9, 12)
                # ---- mid-window: deferred Wo, slot-14 denominator, proj ----
                if pending_wo and w < NS:
                    emit_wo(pending_wo.pop(0))
                if p == NS - 2:
                    # S(14) finished by end of window 14; den mid-window
                    pd_t[p] = pdpool.tile([128, 512], F32, tag="pd", name=f"pden_{p}")
                    S = S_t[p]
                    for g in range(8):
                        nc.tensor.matmul(
                            pd_t[p][:], ONES[:], S[g][:], start=(g == 0), stop=(g == 7)
                        )
                    emit_newton(p)
                for fn in proj_sched.get(w, []):
                    fn()
                if s is not None:
                    emit_qk(s, 5)
                    if s < NS - 1:
                        emit_s(s, 5)
                if w == 0:
                    w0_proj[4]()
                if p >= 0 and p < NS - 1:
                    emit_av(p, 12, 15)
                if s is not None:
                    emit_qk(s, 6)
                    if s < NS - 1:
                        emit_s(s, 6)
                    emit_qk(s, 7)
                    if s < NS - 1:
                        emit_s(s, 7)
                if p >= 0 and p < NS - 1:
                    emit_av(p, 15, 16)
                # ---- bottom: W/den/newton for p first (frees U(p) slots),
                # then the U tree for s ---------------------------------------
                if p >= 0 and p < NS - 2:
                    U = W_t[p]
                    W = [
                        uwpool.tile([128, 512], BF, tag="UW", name=f"W_{p}_{i}")
                        for i in range(2)
                    ]
                    nc.vector.tensor_add(W[0][:], U[0][:], U[1][:])
                    nc.vector.tensor_add(W[1][:], U[2][:], U[3][:])
                    pd_t[p] = pdpool.tile([128, 512], F32, tag="pd", name=f"pden_{p}")
                    nc.tensor.matmul(pd_t[p][:], ONES[:], W[0][:], start=True, stop=False)
                    nc.tensor.matmul(pd_t[p][:], ONES[:], W[1][:], start=False, stop=True)
                    emit_newton(p)
                if s is not None and s < NS - 2:
                    S = S_t[s]
                    U = [
                        uwpool.tile([128, 512], BF, tag="UW", name=f"U_{s}_{i}")
                        for i in range(4)
                    ]
                    nc.gpsimd.tensor_add(U[0][:], S[0][:], S[1][:])
                    nc.gpsimd.tensor_add(U[1][:], S[2][:], S[3][:])
                    nc.vector.tensor_add(U[2][:], S[4][:], S[5][:])
                    nc.vector.tensor_add(U[3][:], S[6][:], S[7][:])
                    W_t[s] = U
                if p >= 0:
                    # normalize (DVE); Wo deferred to the next window's slack
                    emit_mul(p)
                    if slots[p][0] == HPC - 1:
                        pending_wo.append(p)
                    E_t.pop(p)
                    S_t.pop(p, None)
                    W_t.pop(p, None)
            # drain any remaining Wo (xb3) at the very end
            while pending_wo:
                emit_wo(pending_wo.pop(0))

    _split_multi_waits(nc)
    return nc


_NC = None


def _get_nc():
    global _NC
    if _NC is None:
        _NC = _build_nc()
    return _NC


def kernel(x, y, Wq, Wk, Wv, Wo):
    global LAST_EXEC_NS, LAST_RESULTS
    x = np.asarray(x, dtype=np.float32)
    y = np.asarray(y, dtype=np.float32)
    Wq3 = np.asarray(Wq, dtype=np.float32).reshape(H, D, C)
    Wk3 = np.asarray(Wk, dtype=np.float32).reshape(H, D, C)
    Wv3 = np.asarray(Wv, dtype=np.float32).reshape(H, D, C)
    Wo2 = np.asarray(Wo, dtype=np.float32)  # [D, H*D]

    bf16 = ml_dtypes.bfloat16

    in_maps = []
    for core in range(NCORES):
        b, g = core // 2, core % 2
        hsl = slice(4 * g, 4 * g + HPC)
        wqt = Wq3[hsl].reshape(HPC * D, C).T  # [c, hd]
        wkt = Wk3[hsl].reshape(HPC * D, C).T
        wvt = Wv3[hsl].reshape(HPC * D, C).T
        wot = Wo2[:, 4 * g * D : (4 * g + HPC) * D].T  # [hd, o]
        wpack = np.concatenate([wkt, wqt, wvt], axis=1)  # [c, 3*hd]
        in_maps.append(
            {
                "xb": np.ascontiguousarray(x[b]).astype(bf16),
                "yb": np.ascontiguousarray(y[b]).astype(bf16),
                "wpack": np.ascontiguousarray(wpack).astype(bf16),
                "wot": np.ascontiguousarray(wot).astype(bf16),
            }
        )

    import os

    trace = bool(int(os.environ.get("ATTN_TRACE", "0")))
    res = run_bass_kernel_spmd(
        _get_nc(), in_maps, core_ids=list(range(NCORES)), trace=trace
    )
    LAST_EXEC_NS = res.exec_time_ns
    LAST_RESULTS = res

    out = np.empty((B, D, N), dtype=np.float32)
    for b in range(B):
        out[b] = res.results[2 * b]["out"] + res.results[2 * b + 1]["out"]
    return out
